# revision 1
# baseline (speedup 1.0000x reference)
"""StyleGAN2-style conditional generator, data-parallel over batch on 8 NeuronCores.

Self-contained: all shapes/params hardcoded. kernel(**inputs) takes the FULL
unsharded inputs (z, condition, lambda_t, noises, params) and returns the FULL
(8, 3, 256, 256) float32 output. Sharding: batch 8 -> 1 sample per core via
jax.pmap (per-sample modulated conv weights make batch the natural shard axis);
params/noises are replicated. Falls back to single-device jit if pmap on the
neuron cores is unavailable.
"""
import math
import numpy as np
import jax
import jax.numpy as jnp
from jax import lax

B = 8
SIZE = 256
STYLE = 512
LOG = int(math.log2(SIZE))  # 8
NB_VAR = 3
LR_MLP = 0.01
NEG = 0.2
SQRT2 = math.sqrt(2.0)

_blur1d = np.array([1., 3., 3., 1.])
_k = np.outer(_blur1d, _blur1d)
_K2D = np.asarray(_k / _k.sum(), np.float32)


def _eq_lin(x, w, b, lr_mul=1.0, act=False):
    scale = lr_mul / math.sqrt(w.shape[1])
    out = x @ (w * scale).T + b * lr_mul
    if act:
        out = jnp.where(out >= 0, out, NEG * out) * SQRT2
    return out


def _fir(x, k2d, pad, up=1):
    C = x.shape[1]
    kern = jnp.broadcast_to(k2d, (C, 1, 4, 4))
    return lax.conv_general_dilated(
        x, kern, (1, 1), ((pad[0], pad[1]), (pad[0], pad[1])),
        lhs_dilation=(up, up), feature_group_count=C,
        dimension_numbers=('NCHW', 'OIHW', 'NCHW'))


def _modconv3(x, lat, c, k2d_up, up=False):
    O, I, k, _ = c['W'].shape
    scale = 1.0 / math.sqrt(I * k * k)
    s = _eq_lin(lat, c['mw'], c['mb'])
    w = scale * c['W'][None] * s[:, None, :, None, None]
    d = lax.rsqrt(jnp.sum(w * w, axis=(2, 3, 4)) + 1e-8)
    x = x * s[:, :, None, None]
    if up:
        kern = jnp.flip(scale * c['W'], (2, 3))
        out = lax.conv_general_dilated(x, kern, (1, 1), ((k - 1, k - 1),) * 2,
                                       lhs_dilation=(2, 2),
                                       dimension_numbers=('NCHW', 'OIHW', 'NCHW'))
        out = out * d[:, :, None, None]
        out = _fir(out, k2d_up, (1, 1))
    else:
        out = lax.conv_general_dilated(x, scale * c['W'], (1, 1), ((k // 2, k // 2),) * 2,
                                       dimension_numbers=('NCHW', 'OIHW', 'NCHW'))
        out = out * d[:, :, None, None]
    return out


def _styled(x, lat, c, noise, k2d_up, up=False):
    out = _modconv3(x, lat, c, k2d_up, up=up)
    out = out + c['nw'] * noise
    out = out + c['ab'][None, :, None, None]
    return jnp.where(out >= 0, out, NEG * out) * SQRT2


def _torgb(x, lat, c):
    I = c['W'].shape[1]
    s = _eq_lin(lat, c['mw'], c['mb'])
    out = jnp.einsum('bihw,oi->bohw', x * s[:, :, None, None], c['W'] / math.sqrt(I))
    return out + c['bias'][None, :, None, None]


def _forward(z, condition, lambda_t, noises, params, k2d, k2d_up):
    p = params
    y = _eq_lin(condition.reshape(condition.shape[0], -1), p['socket_w'], p['socket_b'])
    s = z * lax.rsqrt(jnp.mean(z * z, axis=-1, keepdims=True) + 1e-8)
    for w, b in p['mlp'][:3]:
        s = _eq_lin(s, w, b, LR_MLP, True)
    lat = s + lambda_t * y
    for w, b in p['mlp'][3:]:
        lat = _eq_lin(lat, w, b, LR_MLP, True)
    Bn = z.shape[0]
    out = jnp.broadcast_to(p['const'], (Bn,) + p['const'].shape[1:])
    out = _styled(out, lat, p['conv1'], noises[0], k2d_up)
    skip = _torgb(out, lat, p['torgb1'])
    ni = 1
    for bi in range(LOG - 2):
        out = _styled(out, lat, p['convs'][2 * bi], noises[ni], k2d_up, up=True)
        out = _styled(out, lat, p['convs'][2 * bi + 1], noises[ni + 1], k2d_up)
        skip = _torgb(out, lat, p['torgbs'][bi]) + _fir(skip, k2d_up, (2, 2), up=2)
        ni += 2
    return skip


_compiled = {}


def kernel(z, condition, lambda_t, noises, params):
    z = np.asarray(z, np.float32)
    condition = np.asarray(condition, np.float32)
    lambda_t = np.asarray(lambda_t, np.float32)
    noises = [np.asarray(n, np.float32) for n in noises]
    params = jax.tree_util.tree_map(lambda a: np.asarray(a, np.float32), params)
    k2d = _K2D
    k2d_up = _K2D * 4.0

    def run_pmap():
        devs = jax.devices()[:8]
        if len(devs) < 8:
            raise RuntimeError('need 8 devices')
        if 'pmap' not in _compiled:
            _compiled['pmap'] = jax.pmap(
                _forward,
                in_axes=(0, 0, None, None, None, None, None),
                static_broadcasted_argnums=(),
                devices=devs)
        f = _compiled['pmap']
        # shard batch 8 -> (8 devices, 1 sample each)
        zs = z.reshape(8, 1, STYLE)
        cs = condition.reshape(8, 1, 3, 32, 32)
        out = f(zs, cs, lambda_t, noises, params, k2d, k2d_up)
        return np.asarray(out).reshape(B, NB_VAR, SIZE, SIZE)

    try:
        return run_pmap().astype(np.float32)
    except Exception:
        if 'jit' not in _compiled:
            cpu = jax.devices('cpu')[0] if jax.devices('cpu') else None
            _compiled['jit'] = jax.jit(_forward, device=cpu) if cpu else jax.jit(_forward)
        out = _compiled['jit'](z, condition, lambda_t, noises, params, k2d, k2d_up)
        return np.asarray(out, np.float32)
